# revision 23
# baseline (speedup 1.0000x reference)
"""Trainium2 Bass kernel for GroupLinear:
    out = einsum('lgi,lgj,ogij->lo', x1, x2, W.reshape(O,g,b,b)) + bias

Equivalent to Y = outer @ W.T + b where outer[l, k] (k = g*b*b + i*b + j) is
the blockwise outer product x1[l,g,i]*x2[l,g,j] -- a [2048, 65536] @
[65536, 1024] matmul whose LHS is generated on the fly.

Sharding: tensor-parallel over the contraction dim. Core c owns weight
blocks g in {2c, 2c+1} (K_local = 8192), computes a full [2048, 1024]
partial, and the host sums the 8 partials (+ bias).

The kernel is PE-bound (~442us of bf16 matmul streaming at the 78.6
TF/s roofline), so two things matter: keeping the PE dense edge-to-edge,
and shaving matmul volume where precision allows.

  - fp8 fraction: the last 2*NF8 of each core's 64 k-chunks run as fp8
    e4m3 DoubleRow matmuls (2 contraction rows/cell -> ~1.8x rate).
    W is too small for e4m3's normal range, so those chunks are scaled
    by 2^13 host-side and accumulated in SEPARATE psum banks; the drain
    folds them in as o += F * 2^-13 (DVE scalar_tensor_tensor). With
    NF8=6 (3/16 of the volume) the deterministic rel err is ~1.6e-2,
    under the 2e-2 gate.
  - Phase 1 (l-blocks 0..2, chunk-outer): 6 psum banks accumulate three
    l-blocks at once, so each W chunk is consumed 6 matmuls at a time
    as it arrives -- PE demand stays under the DMA supply rate while
    the weight shard streams in. Its psum pool closes afterwards and a
    second pool (bf + fp8 accumulators) takes over for phase 2.
  - Startup DMAs: the DMA path is packet-latency-bound for the first
    ~8us (128 packets per transfer, one per partition row), so the
    first transfer is a single contiguous "head" blob carrying all the
    operands the first chunks need; W streams in consumption order in
    small pieces on the sync queue.
  - Output is bf16, shipped as independent half tiles so each DMA
    depends only on its own drain.
"""

import sys
import numpy as np

sys.path.insert(0, "/opt/trn_rl_repo")

import ml_dtypes  # noqa: E402

BF16 = ml_dtypes.bfloat16
F8 = ml_dtypes.float8_e4m3fn

L = 2048
H = 1024
O = 1024
B = 64
G = 16
NCORES = 8
GPC = G // NCORES          # weight blocks per core = 2
KL = GPC * B * B           # local contraction dim = 8192
NCHUNK = KL // 128         # 64 k-chunks of 128
LB = 128                   # l-block (tokens per psum tile)
NLB = L // LB              # 16
R1 = 3                     # l-blocks processed chunk-outer in phase 1
NF8 = 8                    # fp8 DoubleRow pairs (2 chunks each) per core
NBF = NCHUNK - 2 * NF8     # bf16 chunks (0..NBF-1); fp8 covers NBF..63
SW8 = 13                   # W fp8 scale exponent: W*2^13 is e4m3-normal

_cache = {}


def _build_nc():
    from concourse import bass, tile, bacc

    mybir = bass.mybir
    bf = mybir.dt.bfloat16
    f8 = mybir.dt.float8e4
    f32 = mybir.dt.float32
    DR = mybir.MatmulPerfMode.DoubleRow
    SC8 = float(2.0 ** -SW8)

    nc = bacc.Bacc("TRN2", target_bir_lowering=False, debug=False)
    XA = R1 * LB               # x2 columns needed by phase 1 = 384
    HDC = 640                  # head covers chunks 0-4 of each slab
    HD = R1 * HDC + GPC * XA   # head blob: R1 slab-heads + 2 x2 slices
    wp = nc.dram_tensor("wp", [128, NBF * O], bf, kind="ExternalInput")
    wf = nc.dram_tensor("wf", [128, NF8, 2, O], f8, kind="ExternalInput")
    x1r = nc.dram_tensor("x1r", [NLB, 128, KL], bf, kind="ExternalInput")
    x2s = nc.dram_tensor("x2s", [GPC, 128, L], bf, kind="ExternalInput")
    hd = nc.dram_tensor("hd", [128, HD], bf, kind="ExternalInput")
    out = nc.dram_tensor("out", [L, O], bf, kind="ExternalOutput")

    with tile.TileContext(nc) as tc:
        with (
            tc.tile_pool(name="wpool", bufs=1) as wpool,
            tc.tile_pool(name="x2pool", bufs=1) as x2pool,
            tc.tile_pool(name="xpool", bufs=4) as xpool,
            tc.tile_pool(name="opool", bufs=3) as opool,
            tc.tile_pool(name="t8pool", bufs=6) as t8pool,
        ):
            wt = wpool.tile([128, NBF * O], bf)
            wft = wpool.tile([128, NF8, 2, O], f8, tag="wft", name="wft")
            head = x2pool.tile([128, HD], bf, tag="head", name="head")
            x2b = [x2pool.tile([128, L - XA], bf, tag=f"x2b_{g}", name=f"x2b_{g}")
                   for g in range(GPC)]
            xts = [xpool.tile([128, KL], bf, tag="xt", name=f"xt_{r}")
                   for r in range(R1)]

            def xa(g):           # x2 slice [128, XA] for group g inside head
                return head[:, R1 * HDC + g * XA:R1 * HDC + (g + 1) * XA]

            def xslab(xtile, r_head, csl):
                # slab cols csl; phase-1 slabs keep chunks 0-4 in the head
                if r_head is not None and csl.stop <= HDC:
                    return head[:, r_head * HDC + csl.start:r_head * HDC + csl.stop]
                return xtile[:, csl]

            def wpiece(c0, c1, eng):
                sl = slice(c0 * O, c1 * O)
                eng.dma_start(wt[:, sl], wp[:, sl])

            # startup streams, in consumption order, small pieces first.
            # sync carries the weight stream (semaphore granularity must
            # track the PE's chunk-by-chunk consumption in phase 1);
            # scalar carries the head blob + slab pieces + x2 tails.
            W_PIECES = [(0, 1), (1, 2), (2, 4), (4, 6), (6, 8), (8, 10),
                        (10, 12), (12, 16)] + [
                (a, min(a + 4, NBF)) for a in range(16, NBF, 4)
            ]
            SLAB_PIECES = [(HDC, 2048), (2048, 4096), (4096, KL)]
            for c0, c1 in W_PIECES:
                wpiece(c0, c1, nc.sync)
            nc.sync.dma_start(wft[:], wf[:])
            nc.scalar.dma_start(head[:], hd[:])
            for a, b_ in SLAB_PIECES:
                for r in range(R1):
                    nc.scalar.dma_start(xts[r][:, a:b_], x1r[r][:, a:b_])
            for g in range(GPC):
                nc.scalar.dma_start(x2b[g][:], x2s[g][:, XA:L])

            # ---- phase 1a: l-blocks 0..R1-1, bf16 chunks, chunk-outer ----
            oh = []
            with tc.tile_pool(name="psumA", bufs=1, space="PSUM") as psumA:
                ps1 = [
                    [psumA.tile([128, 512], f32, name=f"psA{r}{h}",
                                tag=f"psA{r}{h}", bufs=1) for h in range(2)]
                    for r in range(R1)
                ]
                for c in range(NBF):
                    g = c >> 5
                    csl = slice(c * 128, (c + 1) * 128)
                    first, last = c == 0, c == NBF - 1
                    for r in range(R1):
                        t_ = xslab(xts[r], r, csl)
                        nc.vector.tensor_mul(
                            t_, t_, xa(g)[:, r * LB:(r + 1) * LB],
                        )
                    for r in range(R1):
                        for h in range(2):
                            nc.tensor.matmul(
                                ps1[r][h][:],
                                xslab(xts[r], r, csl),
                                wt[:, c * O + h * 512:c * O + (h + 1) * 512],
                                start=first,
                                stop=last,
                            )
                # drain bf16 partials into output half tiles (the output
                # DMAs wait for the fp8 correction below)
                for r in range(R1):
                    o0 = opool.tile([128, 512], bf, tag="oh0", name="o0")
                    o1 = opool.tile([128, 512], bf, tag="oh1", name="o1")
                    nc.scalar.mul(o0[:], ps1[r][0][:], 1.0)
                    nc.scalar.mul(o1[:], ps1[r][1][:], 1.0)
                    oh.append((o0, o1))

            xt_next = xpool.tile([128, KL], bf, tag="xt", name="xt")
            nc.scalar.dma_start(xt_next[:], x1r[R1])

            with tc.tile_pool(name="psumB", bufs=3, space="PSUM") as psum:
                # ---- phase 1b: fp8 chunks of l-blocks 0..R1-1 ----
                for r in range(R1):
                    F = [psum.tile([128, 512], f32, tag="f8", name="Fp")
                         for _ in range(2)]
                    for j in range(NF8):
                        t8 = t8pool.tile([128, 2, 128], f8, tag="t8", name="t8")
                        for s in range(2):
                            c = NBF + 2 * j + s
                            csl = slice(c * 128, (c + 1) * 128)
                            nc.vector.tensor_mul(
                                t8[:, s, :], xslab(xts[r], r, csl),
                                xa(c >> 5)[:, r * LB:(r + 1) * LB],
                            )
                        for h in range(2):
                            nc.tensor.matmul(
                                F[h][:], t8[:, :, :],
                                wft[:, j, :, h * 512:(h + 1) * 512],
                                start=(j == 0), stop=(j == NF8 - 1),
                                perf_mode=DR,
                            )
                    for h in range(2):
                        o_ = oh[r][h]
                        nc.vector.scalar_tensor_tensor(
                            o_[:], F[h][:], SC8, o_[:],
                            op0=mybir.AluOpType.mult,
                            op1=mybir.AluOpType.add,
                        )
                        nc.sync.dma_start(
                            out[r * LB:(r + 1) * LB, h * 512:(h + 1) * 512],
                            o_[:],
                        )

                # ---- phase 2: l-blocks R1..NLB-1, lb-outer ----
                for lb in range(R1, NLB):
                    xt = xt_next
                    if lb + 1 < NLB:
                        xt_next = xpool.tile([128, KL], bf, tag="xt", name="xt")
                        nc.scalar.dma_start(xt_next[:], x1r[lb + 1])
                    lsl = slice(lb * LB, (lb + 1) * LB)
                    bsl = slice(lb * LB - XA, (lb + 1) * LB - XA)
                    ps0 = psum.tile([128, 512], f32, tag="ps", name="ps0")
                    ps1_ = psum.tile([128, 512], f32, tag="ps", name="ps1")
                    for c in range(NBF):
                        g = c >> 5
                        csl = slice(c * 128, (c + 1) * 128)
                        nc.vector.tensor_mul(xt[:, csl], xt[:, csl],
                                             x2b[g][:, bsl])
                        nc.tensor.matmul(
                            ps0[:], xt[:, csl], wt[:, c * O:c * O + 512],
                            start=(c == 0), stop=(c == NBF - 1),
                        )
                    for c in range(NBF):
                        csl = slice(c * 128, (c + 1) * 128)
                        nc.tensor.matmul(
                            ps1_[:], xt[:, csl],
                            wt[:, c * O + 512:(c + 1) * O],
                            start=(c == 0), stop=(c == NBF - 1),
                        )
                    F = [psum.tile([128, 512], f32, tag="f8", name="Fp")
                         for _ in range(2)]
                    for j in range(NF8):
                        t8 = t8pool.tile([128, 2, 128], f8, tag="t8", name="t8")
                        for s in range(2):
                            c = NBF + 2 * j + s
                            csl = slice(c * 128, (c + 1) * 128)
                            nc.vector.tensor_mul(
                                t8[:, s, :], xt[:, csl],
                                x2b[c >> 5][:, bsl],
                            )
                        for h in range(2):
                            nc.tensor.matmul(
                                F[h][:], t8[:, :, :],
                                wft[:, j, :, h * 512:(h + 1) * 512],
                                start=(j == 0), stop=(j == NF8 - 1),
                                perf_mode=DR,
                            )
                    for h, ps_ in ((0, ps0), (1, ps1_)):
                        o_ = opool.tile([128, 512], bf, tag=f"oh{h}",
                                        name="oo")
                        nc.scalar.mul(o_[:], ps_[:], 1.0)
                        nc.vector.scalar_tensor_tensor(
                            o_[:], F[h][:], SC8, o_[:],
                            op0=mybir.AluOpType.mult,
                            op1=mybir.AluOpType.add,
                        )
                        nc.sync.dma_start(
                            out[lsl, h * 512:(h + 1) * 512], o_[:],
                        )

    nc.compile()
    return nc


def _prep_inputs(input1, input2, W):
    """Host-side shard + layout (transposes / gathers / dtype casts only)."""
    x1 = np.ascontiguousarray(input1, dtype=np.float32)
    x2 = np.ascontiguousarray(input2, dtype=np.float32)
    Wt = np.ascontiguousarray(W.T, dtype=np.float32)  # [65536, 1024], k-major

    in_maps = []
    for core in range(NCORES):
        ks = slice(core * KL, (core + 1) * KL)
        gs = slice(core * GPC, (core + 1) * GPC)
        Wk = Wt[ks].reshape(NCHUNK, 128, O)
        # bf16 chunks 0..NBF-1: [c, p, o] -> [p, c*O + o]
        wp = (
            Wk[:NBF]
            .transpose(1, 0, 2)
            .reshape(128, NBF * O)
            .astype(BF16)
        )
        # fp8 chunks NBF..63, DoubleRow pairs: [p, j, s, o], scaled 2^SW8
        wf = (
            (Wk[NBF:] * float(2.0 ** SW8))
            .reshape(NF8, 2, 128, O)
            .transpose(2, 0, 1, 3)
            .astype(F8)
        )
        # x1 replicated over j: k_local = g*B*B + i*B + j -> x1[l, g, i]
        x1g = x1.reshape(L, G, B)[:, gs, :].transpose(1, 2, 0)  # [g, i, l]
        rep = np.repeat(x1g, B, axis=1).reshape(KL, L)          # [k_local, l]
        x1r = (
            rep.reshape(NCHUNK, 128, NLB, LB)
            .transpose(2, 1, 0, 3)
            .reshape(NLB, 128, KL)
            .astype(BF16)
        )
        # x2 stacked twice along partitions: row p -> j = p % 64
        x2g = x2.reshape(L, G, B)[:, gs, :].transpose(1, 2, 0)  # [g, j, l]
        x2st = np.concatenate([x2g, x2g], axis=1).astype(BF16)  # [g, 128, l]
        hd = np.concatenate(
            [x1r[r, :, 0:640] for r in range(R1)]
            + [x2st[g][:, 0:R1 * 128] for g in range(GPC)],
            axis=1,
        )
        in_maps.append(
            {
                "wp": np.ascontiguousarray(wp),
                "wf": np.ascontiguousarray(wf),
                "x1r": np.ascontiguousarray(x1r),
                "x2s": np.ascontiguousarray(x2st),
                "hd": np.ascontiguousarray(hd),
            }
        )
    return in_maps


def run(input1, input2, W, b, trace=False, tmpdir=None):
    """Shard, run on 8 NeuronCores, unshard. Returns (out, BassKernelResults)."""
    from concourse.bass_utils import run_bass_kernel_spmd

    if "nc" not in _cache:
        _cache["nc"] = _build_nc()
    nc = _cache["nc"]

    in_maps = _prep_inputs(input1, input2, W)
    res = run_bass_kernel_spmd(
        nc, in_maps, list(range(NCORES)), trace=trace, tmpdir=tmpdir
    )
    acc = np.zeros((L, O), dtype=np.float32)
    for core in range(NCORES):
        acc += res.results[core]["out"].astype(np.float32)
    acc += np.asarray(b, dtype=np.float32)[None, :]
    return acc, res


def kernel(input1, input2, W, b):
    out, _ = run(input1, input2, W, b, trace=False)
    return out


if __name__ == "__main__":
    rng = np.random.default_rng(0)
    x1 = rng.standard_normal((L, H), dtype=np.float32)
    x2 = rng.standard_normal((L, H), dtype=np.float32)
    W = rng.standard_normal((O, H * B), dtype=np.float32) / 256.0
    b = rng.standard_normal((O,), dtype=np.float32) / 256.0
    out = kernel(x1, x2, W, b)
    print("out", out.shape, out.dtype, float(np.abs(out).max()))


# revision 25
# speedup vs baseline: 1.0064x; 1.0064x over previous
"""Trainium2 Bass kernel for GroupLinear:
    out = einsum('lgi,lgj,ogij->lo', x1, x2, W.reshape(O,g,b,b)) + bias

Equivalent to Y = outer @ W.T + b where outer[l, k] (k = g*b*b + i*b + j) is
the blockwise outer product x1[l,g,i]*x2[l,g,j] -- a [2048, 65536] @
[65536, 1024] matmul whose LHS is generated on the fly.

Sharding: tensor-parallel over the contraction dim. Core c owns weight
blocks g in {2c, 2c+1} (K_local = 8192), computes a full [2048, 1024]
partial, and the host sums the 8 partials (+ bias).

The kernel is PE-bound (~442us of bf16 matmul streaming at the 78.6
TF/s roofline), so two things matter: keeping the PE dense edge-to-edge,
and shaving matmul volume where precision allows.

  - fp8 fraction: the last 2*NF8 of each core's 64 k-chunks run as fp8
    e4m3 DoubleRow matmuls (2 contraction rows/cell -> ~1.8x rate).
    W is too small for e4m3's normal range, so those chunks are scaled
    by 2^13 host-side and accumulated in SEPARATE psum banks; the drain
    folds them in as o += F * 2^-13 (DVE scalar_tensor_tensor). With
    NF8=6 (3/16 of the volume) the deterministic rel err is ~1.6e-2,
    under the 2e-2 gate.
  - Phase 1 (l-blocks 0..2, chunk-outer): 6 psum banks accumulate three
    l-blocks at once, so each W chunk is consumed 6 matmuls at a time
    as it arrives -- PE demand stays under the DMA supply rate while
    the weight shard streams in. Its psum pool closes afterwards and a
    second pool (bf + fp8 accumulators) takes over for phase 2.
  - Startup DMAs: the DMA path is packet-latency-bound for the first
    ~8us (128 packets per transfer, one per partition row), so the
    first transfer is a single contiguous "head" blob carrying all the
    operands the first chunks need; W streams in consumption order in
    small pieces on the sync queue.
  - Output is bf16, shipped as independent half tiles so each DMA
    depends only on its own drain.
"""

import sys
import numpy as np

sys.path.insert(0, "/opt/trn_rl_repo")

import ml_dtypes  # noqa: E402

BF16 = ml_dtypes.bfloat16
F8 = ml_dtypes.float8_e4m3fn

L = 2048
H = 1024
O = 1024
B = 64
G = 16
NCORES = 8
GPC = G // NCORES          # weight blocks per core = 2
KL = GPC * B * B           # local contraction dim = 8192
NCHUNK = KL // 128         # 64 k-chunks of 128
LB = 128                   # l-block (tokens per psum tile)
NLB = L // LB              # 16
R1 = 3                     # l-blocks processed chunk-outer in phase 1
NF8 = 8                    # fp8 DoubleRow pairs (2 chunks each) per core
NBF = NCHUNK - 2 * NF8     # bf16 chunks (0..NBF-1); fp8 covers NBF..63
SW8 = 13                   # W fp8 scale exponent: W*2^13 is e4m3-normal

_cache = {}


def _build_nc():
    from concourse import bass, tile, bacc

    mybir = bass.mybir
    bf = mybir.dt.bfloat16
    f8 = mybir.dt.float8e4
    f32 = mybir.dt.float32
    DR = mybir.MatmulPerfMode.DoubleRow
    SC8 = float(2.0 ** -SW8)

    nc = bacc.Bacc("TRN2", target_bir_lowering=False, debug=False)
    XA = R1 * LB               # x2 columns needed by phase 1 = 384
    HDC = 640                  # head covers chunks 0-4 of each slab
    HD = R1 * HDC + GPC * XA   # head blob: R1 slab-heads + 2 x2 slices
    wp = nc.dram_tensor("wp", [128, NBF * O], bf, kind="ExternalInput")
    wf = nc.dram_tensor("wf", [128, NF8, 2, O], f8, kind="ExternalInput")
    x1r = nc.dram_tensor("x1r", [NLB, 128, KL], bf, kind="ExternalInput")
    x2s = nc.dram_tensor("x2s", [GPC, 128, L], bf, kind="ExternalInput")
    hd = nc.dram_tensor("hd", [128, HD], bf, kind="ExternalInput")
    out = nc.dram_tensor("out", [L, O], bf, kind="ExternalOutput")

    with tile.TileContext(nc) as tc:
        with (
            tc.tile_pool(name="wpool", bufs=1) as wpool,
            tc.tile_pool(name="x2pool", bufs=1) as x2pool,
            tc.tile_pool(name="xpool", bufs=4) as xpool,
            tc.tile_pool(name="opool", bufs=3) as opool,
            tc.tile_pool(name="t8pool", bufs=6) as t8pool,
        ):
            wt = wpool.tile([128, NBF * O], bf)
            wft = wpool.tile([128, NF8, 2, O], f8, tag="wft", name="wft")
            head = x2pool.tile([128, HD], bf, tag="head", name="head")
            x2b = [x2pool.tile([128, L - XA], bf, tag=f"x2b_{g}", name=f"x2b_{g}")
                   for g in range(GPC)]
            xts = [xpool.tile([128, KL], bf, tag="xt", name=f"xt_{r}")
                   for r in range(R1)]

            def xa(g):           # x2 slice [128, XA] for group g inside head
                return head[:, R1 * HDC + g * XA:R1 * HDC + (g + 1) * XA]

            def xslab(xtile, r_head, csl):
                # slab cols csl; phase-1 slabs keep chunks 0-4 in the head
                if r_head is not None and csl.stop <= HDC:
                    return head[:, r_head * HDC + csl.start:r_head * HDC + csl.stop]
                return xtile[:, csl]

            def wpiece(c0, c1, eng):
                sl = slice(c0 * O, c1 * O)
                eng.dma_start(wt[:, sl], wp[:, sl])

            # startup streams, in consumption order, small pieces first.
            # sync carries the weight stream (semaphore granularity must
            # track the PE's chunk-by-chunk consumption in phase 1);
            # scalar carries the head blob + slab pieces + x2 tails.
            W_PIECES = [(0, 1), (1, 2), (2, 4), (4, 6), (6, 8), (8, 10),
                        (10, 12), (12, 16)] + [
                (a, min(a + 4, NBF)) for a in range(16, NBF, 4)
            ]
            SLAB_PIECES = [(HDC, 2048), (2048, 4096), (4096, KL)]
            for c0, c1 in W_PIECES:
                wpiece(c0, c1, nc.sync)
            nc.sync.dma_start(wft[:], wf[:])
            nc.scalar.dma_start(head[:], hd[:])
            for a, b_ in SLAB_PIECES:
                for r in range(R1):
                    nc.scalar.dma_start(xts[r][:, a:b_], x1r[r][:, a:b_])
            for g in range(GPC):
                nc.scalar.dma_start(x2b[g][:], x2s[g][:, XA:L])

            # ---- phase 1a: l-blocks 0..R1-1, bf16 chunks, chunk-outer ----
            # one psum pool for everything: phase 1a borrows 3 slots from
            # each of the two phase-2 tag rings, so there is no pool
            # handover barrier between the phases
            _psum_cm = tc.tile_pool(name="psum", bufs=4, space="PSUM")
            psum = _psum_cm.__enter__()
            oh = []
            if True:
                ps1 = [
                    [psum.tile([128, 512], f32, name=f"psA{r}{h}",
                               tag=("ps", "f8")[h], bufs=4) for h in range(2)]
                    for r in range(R1)
                ]
                for c in range(NBF):
                    g = c >> 5
                    csl = slice(c * 128, (c + 1) * 128)
                    first, last = c == 0, c == NBF - 1
                    for r in range(R1):
                        t_ = xslab(xts[r], r, csl)
                        nc.vector.tensor_mul(
                            t_, t_, xa(g)[:, r * LB:(r + 1) * LB],
                        )
                    for r in range(R1):
                        for h in range(2):
                            nc.tensor.matmul(
                                ps1[r][h][:],
                                xslab(xts[r], r, csl),
                                wt[:, c * O + h * 512:c * O + (h + 1) * 512],
                                start=first,
                                stop=last,
                            )
                # drain bf16 partials into output half tiles (the output
                # DMAs wait for the fp8 correction below)
                for r in range(R1):
                    o0 = opool.tile([128, 512], bf, tag="oh0", name="o0")
                    o1 = opool.tile([128, 512], bf, tag="oh1", name="o1")
                    nc.scalar.mul(o0[:], ps1[r][0][:], 1.0)
                    nc.scalar.mul(o1[:], ps1[r][1][:], 1.0)
                    oh.append((o0, o1))

            xt_next = xpool.tile([128, KL], bf, tag="xt", name="xt")
            nc.scalar.dma_start(xt_next[:], x1r[R1])

            if True:
                # ---- phase 1b: fp8 chunks of l-blocks 0..R1-1 ----
                for r in range(R1):
                    F = [psum.tile([128, 512], f32, tag="f8", name="Fp",
                                   bufs=4)
                         for _ in range(2)]
                    for j in range(NF8):
                        t8 = t8pool.tile([128, 2, 128], f8, tag="t8", name="t8")
                        for s in range(2):
                            c = NBF + 2 * j + s
                            csl = slice(c * 128, (c + 1) * 128)
                            nc.vector.tensor_mul(
                                t8[:, s, :], xslab(xts[r], r, csl),
                                xa(c >> 5)[:, r * LB:(r + 1) * LB],
                            )
                        for h in range(2):
                            nc.tensor.matmul(
                                F[h][:], t8[:, :, :],
                                wft[:, j, :, h * 512:(h + 1) * 512],
                                start=(j == 0), stop=(j == NF8 - 1),
                                perf_mode=DR,
                            )
                    for h in range(2):
                        o_ = oh[r][h]
                        nc.vector.scalar_tensor_tensor(
                            o_[:], F[h][:], SC8, o_[:],
                            op0=mybir.AluOpType.mult,
                            op1=mybir.AluOpType.add,
                        )
                        nc.sync.dma_start(
                            out[r * LB:(r + 1) * LB, h * 512:(h + 1) * 512],
                            o_[:],
                        )

                # ---- phase 2: l-blocks R1..NLB-1, lb-outer ----
                for lb in range(R1, NLB):
                    xt = xt_next
                    if lb + 1 < NLB:
                        xt_next = xpool.tile([128, KL], bf, tag="xt", name="xt")
                        nc.scalar.dma_start(xt_next[:], x1r[lb + 1])
                    lsl = slice(lb * LB, (lb + 1) * LB)
                    bsl = slice(lb * LB - XA, (lb + 1) * LB - XA)
                    last_lb = lb == NLB - 1

                    def f8_pass():
                        F = [psum.tile([128, 512], f32, tag="f8", name="Fp",
                                       bufs=4) for _ in range(2)]
                        for j in range(NF8):
                            t8 = t8pool.tile([128, 2, 128], f8, tag="t8",
                                             name="t8")
                            for s in range(2):
                                c = NBF + 2 * j + s
                                csl = slice(c * 128, (c + 1) * 128)
                                nc.vector.tensor_mul(
                                    t8[:, s, :], xt[:, csl],
                                    x2b[c >> 5][:, bsl],
                                )
                            for h in range(2):
                                nc.tensor.matmul(
                                    F[h][:], t8[:, :, :],
                                    wft[:, j, :, h * 512:(h + 1) * 512],
                                    start=(j == 0), stop=(j == NF8 - 1),
                                    perf_mode=DR,
                                )
                        return F

                    def bf_pass(ps_, h):
                        for c in range(NBF):
                            csl = slice(c * 128, (c + 1) * 128)
                            if h == 0:
                                nc.vector.tensor_mul(xt[:, csl], xt[:, csl],
                                                     x2b[c >> 5][:, bsl])
                            nc.tensor.matmul(
                                ps_[:], xt[:, csl],
                                wt[:, c * O + h * 512:c * O + (h + 1) * 512],
                                start=(c == 0), stop=(c == NBF - 1),
                            )

                    def drain(h, ps_, F):
                        o_ = opool.tile([128, 512], bf, tag=f"oh{h}",
                                        name="oo")
                        nc.scalar.mul(o_[:], ps_[:], 1.0)
                        nc.vector.scalar_tensor_tensor(
                            o_[:], F[h][:], SC8, o_[:],
                            op0=mybir.AluOpType.mult,
                            op1=mybir.AluOpType.add,
                        )
                        nc.sync.dma_start(
                            out[lsl, h * 512:(h + 1) * 512], o_[:],
                        )

                    ps0 = psum.tile([128, 512], f32, tag="ps", name="ps0",
                                    bufs=4)
                    ps1_ = psum.tile([128, 512], f32, tag="ps", name="ps1",
                                     bufs=4)
                    if last_lb:
                        # fp8 first: its t8 muls read the raw slab, so they
                        # must precede the in-place bf16 muls of chunk c<NBF
                        F = f8_pass()
                        bf_pass(ps0, 0)
                        drain(0, ps0, F)
                        bf_pass(ps1_, 1)
                        drain(1, ps1_, F)
                    else:
                        bf_pass(ps0, 0)
                        bf_pass(ps1_, 1)
                        F = f8_pass()
                        drain(0, ps0, F)
                        drain(1, ps1_, F)

            _psum_cm.__exit__(None, None, None)

    nc.compile()
    return nc


def _prep_inputs(input1, input2, W):
    """Host-side shard + layout (transposes / gathers / dtype casts only)."""
    x1 = np.ascontiguousarray(input1, dtype=np.float32)
    x2 = np.ascontiguousarray(input2, dtype=np.float32)
    Wt = np.ascontiguousarray(W.T, dtype=np.float32)  # [65536, 1024], k-major

    in_maps = []
    for core in range(NCORES):
        ks = slice(core * KL, (core + 1) * KL)
        gs = slice(core * GPC, (core + 1) * GPC)
        Wk = Wt[ks].reshape(NCHUNK, 128, O)
        # bf16 chunks 0..NBF-1: [c, p, o] -> [p, c*O + o]
        wp = (
            Wk[:NBF]
            .transpose(1, 0, 2)
            .reshape(128, NBF * O)
            .astype(BF16)
        )
        # fp8 chunks NBF..63, DoubleRow pairs: [p, j, s, o], scaled 2^SW8
        wf = (
            (Wk[NBF:] * float(2.0 ** SW8))
            .reshape(NF8, 2, 128, O)
            .transpose(2, 0, 1, 3)
            .astype(F8)
        )
        # x1 replicated over j: k_local = g*B*B + i*B + j -> x1[l, g, i]
        x1g = x1.reshape(L, G, B)[:, gs, :].transpose(1, 2, 0)  # [g, i, l]
        rep = np.repeat(x1g, B, axis=1).reshape(KL, L)          # [k_local, l]
        x1r = (
            rep.reshape(NCHUNK, 128, NLB, LB)
            .transpose(2, 1, 0, 3)
            .reshape(NLB, 128, KL)
            .astype(BF16)
        )
        # x2 stacked twice along partitions: row p -> j = p % 64
        x2g = x2.reshape(L, G, B)[:, gs, :].transpose(1, 2, 0)  # [g, j, l]
        x2st = np.concatenate([x2g, x2g], axis=1).astype(BF16)  # [g, 128, l]
        hd = np.concatenate(
            [x1r[r, :, 0:640] for r in range(R1)]
            + [x2st[g][:, 0:R1 * 128] for g in range(GPC)],
            axis=1,
        )
        in_maps.append(
            {
                "wp": np.ascontiguousarray(wp),
                "wf": np.ascontiguousarray(wf),
                "x1r": np.ascontiguousarray(x1r),
                "x2s": np.ascontiguousarray(x2st),
                "hd": np.ascontiguousarray(hd),
            }
        )
    return in_maps


def run(input1, input2, W, b, trace=False, tmpdir=None):
    """Shard, run on 8 NeuronCores, unshard. Returns (out, BassKernelResults)."""
    from concourse.bass_utils import run_bass_kernel_spmd

    if "nc" not in _cache:
        _cache["nc"] = _build_nc()
    nc = _cache["nc"]

    in_maps = _prep_inputs(input1, input2, W)
    res = run_bass_kernel_spmd(
        nc, in_maps, list(range(NCORES)), trace=trace, tmpdir=tmpdir
    )
    acc = np.zeros((L, O), dtype=np.float32)
    for core in range(NCORES):
        acc += res.results[core]["out"].astype(np.float32)
    acc += np.asarray(b, dtype=np.float32)[None, :]
    return acc, res


def kernel(input1, input2, W, b):
    out, _ = run(input1, input2, W, b, trace=False)
    return out


if __name__ == "__main__":
    rng = np.random.default_rng(0)
    x1 = rng.standard_normal((L, H), dtype=np.float32)
    x2 = rng.standard_normal((L, H), dtype=np.float32)
    W = rng.standard_normal((O, H * B), dtype=np.float32) / 256.0
    b = rng.standard_normal((O,), dtype=np.float32) / 256.0
    out = kernel(x1, x2, W, b)
    print("out", out.shape, out.dtype, float(np.abs(out).max()))


# revision 26
# speedup vs baseline: 1.0114x; 1.0050x over previous
"""Trainium2 Bass kernel for GroupLinear:
    out = einsum('lgi,lgj,ogij->lo', x1, x2, W.reshape(O,g,b,b)) + bias

Equivalent to Y = outer @ W.T + b where outer[l, k] (k = g*b*b + i*b + j) is
the blockwise outer product x1[l,g,i]*x2[l,g,j] -- a [2048, 65536] @
[65536, 1024] matmul whose LHS is generated on the fly.

Sharding: tensor-parallel over the contraction dim. Core c owns weight
blocks g in {2c, 2c+1} (K_local = 8192), computes a full [2048, 1024]
partial, and the host sums the 8 partials (+ bias).

The kernel is PE-bound (~442us of bf16 matmul streaming at the 78.6
TF/s roofline), so two things matter: keeping the PE dense edge-to-edge,
and shaving matmul volume where precision allows.

  - fp8 fraction: the last 2*NF8 of each core's 64 k-chunks run as fp8
    e4m3 DoubleRow matmuls (2 contraction rows/cell -> ~1.8x rate).
    W is too small for e4m3's normal range, so those chunks are scaled
    by 2^13 host-side and accumulated in SEPARATE psum banks; the drain
    folds them in as o += F * 2^-13 (DVE scalar_tensor_tensor). With
    NF8=6 (3/16 of the volume) the deterministic rel err is ~1.6e-2,
    under the 2e-2 gate.
  - Phase 1 (l-blocks 0..2, chunk-outer): 6 psum banks accumulate three
    l-blocks at once, so each W chunk is consumed 6 matmuls at a time
    as it arrives -- PE demand stays under the DMA supply rate while
    the weight shard streams in. Its psum pool closes afterwards and a
    second pool (bf + fp8 accumulators) takes over for phase 2.
  - Startup DMAs: the DMA path is packet-latency-bound for the first
    ~8us (128 packets per transfer, one per partition row), so the
    first transfer is a single contiguous "head" blob carrying all the
    operands the first chunks need; W streams in consumption order in
    small pieces on the sync queue.
  - Output is bf16, shipped as independent half tiles so each DMA
    depends only on its own drain.
"""

import sys
import numpy as np

sys.path.insert(0, "/opt/trn_rl_repo")

import ml_dtypes  # noqa: E402

BF16 = ml_dtypes.bfloat16
F8 = ml_dtypes.float8_e4m3fn

L = 2048
H = 1024
O = 1024
B = 64
G = 16
NCORES = 8
GPC = G // NCORES          # weight blocks per core = 2
KL = GPC * B * B           # local contraction dim = 8192
NCHUNK = KL // 128         # 64 k-chunks of 128
LB = 128                   # l-block (tokens per psum tile)
NLB = L // LB              # 16
R1 = 3                     # l-blocks processed chunk-outer in phase 1
NF8 = 8                    # fp8 DoubleRow pairs (2 chunks each) per core
NBF = NCHUNK - 2 * NF8     # bf16 chunks (0..NBF-1); fp8 covers NBF..63
SW8 = 13                   # W fp8 scale exponent: W*2^13 is e4m3-normal

_cache = {}


def _build_nc():
    from concourse import bass, tile, bacc

    mybir = bass.mybir
    bf = mybir.dt.bfloat16
    f8 = mybir.dt.float8e4
    f32 = mybir.dt.float32
    DR = mybir.MatmulPerfMode.DoubleRow
    SC8 = float(2.0 ** -SW8)

    nc = bacc.Bacc("TRN2", target_bir_lowering=False, debug=False)
    XA = R1 * LB               # x2 columns needed by phase 1 = 384
    HDC = 640                  # heads cover chunks 0-4 of each slab
    HD0 = R1 * 128 + XA        # head0: chunk 0 of each slab + x2 for g=0
    HD1 = R1 * 512 + XA        # head1: chunks 1-4 of each slab + x2 for g=1
    wp = nc.dram_tensor("wp", [128, NBF * O], bf, kind="ExternalInput")
    wf = nc.dram_tensor("wf", [128, NF8, 2, O], f8, kind="ExternalInput")
    x1r = nc.dram_tensor("x1r", [NLB, 128, KL], bf, kind="ExternalInput")
    x2s = nc.dram_tensor("x2s", [GPC, 128, L], bf, kind="ExternalInput")
    hd0 = nc.dram_tensor("hd0", [128, HD0], bf, kind="ExternalInput")
    hd1 = nc.dram_tensor("hd1", [128, HD1], bf, kind="ExternalInput")
    out = nc.dram_tensor("out", [L, O], bf, kind="ExternalOutput")

    with tile.TileContext(nc) as tc:
        with (
            tc.tile_pool(name="wpool", bufs=1) as wpool,
            tc.tile_pool(name="x2pool", bufs=1) as x2pool,
            tc.tile_pool(name="xpool", bufs=4) as xpool,
            tc.tile_pool(name="opool", bufs=3) as opool,
            tc.tile_pool(name="t8pool", bufs=6) as t8pool,
        ):
            wt = wpool.tile([128, NBF * O], bf)
            wft = wpool.tile([128, NF8, 2, O], f8, tag="wft", name="wft")
            head0 = x2pool.tile([128, HD0], bf, tag="head0", name="head0")
            head1 = x2pool.tile([128, HD1], bf, tag="head1", name="head1")
            x2b = [x2pool.tile([128, L - XA], bf, tag=f"x2b_{g}", name=f"x2b_{g}")
                   for g in range(GPC)]
            xts = [xpool.tile([128, KL], bf, tag="xt", name=f"xt_{r}")
                   for r in range(R1)]

            def xa(g):           # x2 slice [128, XA] for group g, in heads
                hh, base = (head0, R1 * 128) if g == 0 else (head1, R1 * 512)
                return hh[:, base:base + XA]

            def xslab(xtile, r_head, csl):
                # slab cols csl; phase-1 slabs keep chunks 0-4 in the heads
                if r_head is not None and csl.stop <= 128:
                    return head0[:, r_head * 128 + csl.start:
                                  r_head * 128 + csl.stop]
                if r_head is not None and csl.stop <= HDC:
                    a = r_head * 512 + csl.start - 128
                    return head1[:, a:a + 128]
                return xtile[:, csl]

            def wpiece(c0, c1, eng):
                sl = slice(c0 * O, c1 * O)
                eng.dma_start(wt[:, sl], wp[:, sl])

            # startup streams, in consumption order, small pieces first.
            # sync carries the weight stream (semaphore granularity must
            # track the PE's chunk-by-chunk consumption in phase 1);
            # scalar carries the head blob + slab pieces + x2 tails.
            W_PIECES = [(0, 1), (1, 2), (2, 3), (3, 4)] + [
                (a, a + 2) for a in range(4, 24, 2)
            ] + [(a, min(a + 4, NBF)) for a in range(24, NBF, 4)]
            SLAB_PIECES = [(HDC, 2048), (2048, 4096), (4096, KL)]
            for c0, c1 in W_PIECES:
                wpiece(c0, c1, nc.sync)
            nc.sync.dma_start(wft[:], wf[:])
            nc.scalar.dma_start(head0[:], hd0[:])
            nc.scalar.dma_start(head1[:], hd1[:])
            for a, b_ in SLAB_PIECES:
                for r in range(R1):
                    nc.scalar.dma_start(xts[r][:, a:b_], x1r[r][:, a:b_])
            for g in range(GPC):
                nc.scalar.dma_start(x2b[g][:], x2s[g][:, XA:L])

            # ---- phase 1a: l-blocks 0..R1-1, bf16 chunks, chunk-outer ----
            # one psum pool for everything: phase 1a borrows 3 slots from
            # each of the two phase-2 tag rings, so there is no pool
            # handover barrier between the phases
            _psum_cm = tc.tile_pool(name="psum", bufs=4, space="PSUM")
            psum = _psum_cm.__enter__()
            oh = []
            if True:
                ps1 = [
                    [psum.tile([128, 512], f32, name=f"psA{r}{h}",
                               tag=("ps", "f8")[h], bufs=4) for h in range(2)]
                    for r in range(R1)
                ]
                for c in range(NBF):
                    g = c >> 5
                    csl = slice(c * 128, (c + 1) * 128)
                    first, last = c == 0, c == NBF - 1
                    for r in range(R1):
                        t_ = xslab(xts[r], r, csl)
                        nc.vector.tensor_mul(
                            t_, t_, xa(g)[:, r * LB:(r + 1) * LB],
                        )
                    for r in range(R1):
                        for h in range(2):
                            nc.tensor.matmul(
                                ps1[r][h][:],
                                xslab(xts[r], r, csl),
                                wt[:, c * O + h * 512:c * O + (h + 1) * 512],
                                start=first,
                                stop=last,
                            )
                # drain bf16 partials into output half tiles (the output
                # DMAs wait for the fp8 correction below)
                for r in range(R1):
                    o0 = opool.tile([128, 512], bf, tag="oh0", name="o0")
                    o1 = opool.tile([128, 512], bf, tag="oh1", name="o1")
                    nc.scalar.mul(o0[:], ps1[r][0][:], 1.0)
                    nc.scalar.mul(o1[:], ps1[r][1][:], 1.0)
                    oh.append((o0, o1))

            xt_next = xpool.tile([128, KL], bf, tag="xt", name="xt")
            nc.scalar.dma_start(xt_next[:], x1r[R1])

            if True:
                # ---- phase 1b: fp8 chunks of l-blocks 0..R1-1 ----
                for r in range(R1):
                    F = [psum.tile([128, 512], f32, tag="f8", name="Fp",
                                   bufs=4)
                         for _ in range(2)]
                    for j in range(NF8):
                        t8 = t8pool.tile([128, 2, 128], f8, tag="t8", name="t8")
                        for s in range(2):
                            c = NBF + 2 * j + s
                            csl = slice(c * 128, (c + 1) * 128)
                            nc.vector.tensor_mul(
                                t8[:, s, :], xslab(xts[r], r, csl),
                                xa(c >> 5)[:, r * LB:(r + 1) * LB],
                            )
                        for h in range(2):
                            nc.tensor.matmul(
                                F[h][:], t8[:, :, :],
                                wft[:, j, :, h * 512:(h + 1) * 512],
                                start=(j == 0), stop=(j == NF8 - 1),
                                perf_mode=DR,
                            )
                    for h in range(2):
                        o_ = oh[r][h]
                        nc.vector.scalar_tensor_tensor(
                            o_[:], F[h][:], SC8, o_[:],
                            op0=mybir.AluOpType.mult,
                            op1=mybir.AluOpType.add,
                        )
                        nc.sync.dma_start(
                            out[r * LB:(r + 1) * LB, h * 512:(h + 1) * 512],
                            o_[:],
                        )

                # ---- phase 2: l-blocks R1..NLB-1, lb-outer ----
                for lb in range(R1, NLB):
                    xt = xt_next
                    if lb + 1 < NLB:
                        xt_next = xpool.tile([128, KL], bf, tag="xt", name="xt")
                        nc.scalar.dma_start(xt_next[:], x1r[lb + 1])
                    lsl = slice(lb * LB, (lb + 1) * LB)
                    bsl = slice(lb * LB - XA, (lb + 1) * LB - XA)
                    last_lb = lb == NLB - 1

                    def f8_pass():
                        F = [psum.tile([128, 512], f32, tag="f8", name="Fp",
                                       bufs=4) for _ in range(2)]
                        for j in range(NF8):
                            t8 = t8pool.tile([128, 2, 128], f8, tag="t8",
                                             name="t8")
                            for s in range(2):
                                c = NBF + 2 * j + s
                                csl = slice(c * 128, (c + 1) * 128)
                                nc.vector.tensor_mul(
                                    t8[:, s, :], xt[:, csl],
                                    x2b[c >> 5][:, bsl],
                                )
                            for h in range(2):
                                nc.tensor.matmul(
                                    F[h][:], t8[:, :, :],
                                    wft[:, j, :, h * 512:(h + 1) * 512],
                                    start=(j == 0), stop=(j == NF8 - 1),
                                    perf_mode=DR,
                                )
                        return F

                    def bf_pass(ps_, h):
                        for c in range(NBF):
                            csl = slice(c * 128, (c + 1) * 128)
                            if h == 0:
                                nc.vector.tensor_mul(xt[:, csl], xt[:, csl],
                                                     x2b[c >> 5][:, bsl])
                            nc.tensor.matmul(
                                ps_[:], xt[:, csl],
                                wt[:, c * O + h * 512:c * O + (h + 1) * 512],
                                start=(c == 0), stop=(c == NBF - 1),
                            )

                    def drain(h, ps_, F):
                        o_ = opool.tile([128, 512], bf, tag=f"oh{h}",
                                        name="oo")
                        nc.scalar.mul(o_[:], ps_[:], 1.0)
                        nc.vector.scalar_tensor_tensor(
                            o_[:], F[h][:], SC8, o_[:],
                            op0=mybir.AluOpType.mult,
                            op1=mybir.AluOpType.add,
                        )
                        nc.sync.dma_start(
                            out[lsl, h * 512:(h + 1) * 512], o_[:],
                        )

                    ps0 = psum.tile([128, 512], f32, tag="ps", name="ps0",
                                    bufs=4)
                    ps1_ = psum.tile([128, 512], f32, tag="ps", name="ps1",
                                     bufs=4)
                    if last_lb:
                        # fp8 first: its t8 muls read the raw slab, so they
                        # must precede the in-place bf16 muls of chunk c<NBF.
                        # h=1 runs as two sequential quarter passes so the
                        # final drain + output DMA are half-sized.
                        F = f8_pass()
                        bf_pass(ps0, 0)
                        drain(0, ps0, F)
                        for q, (q0, q1) in enumerate([(512, 768), (768, O)]):
                            psq = psum.tile([128, 256], f32, tag="ps",
                                            name="psq", bufs=4)
                            for c in range(NBF):
                                csl = slice(c * 128, (c + 1) * 128)
                                nc.tensor.matmul(
                                    psq[:], xt[:, csl],
                                    wt[:, c * O + q0:c * O + q1],
                                    start=(c == 0), stop=(c == NBF - 1),
                                )
                            oq = opool.tile([128, 256], bf, tag="oh1",
                                            name="oq")
                            nc.scalar.mul(oq[:], psq[:], 1.0)
                            nc.vector.scalar_tensor_tensor(
                                oq[:], F[1][:, q0 - 512:q1 - 512], SC8, oq[:],
                                op0=mybir.AluOpType.mult,
                                op1=mybir.AluOpType.add,
                            )
                            nc.sync.dma_start(out[lsl, q0:q1], oq[:])
                    else:
                        bf_pass(ps0, 0)
                        bf_pass(ps1_, 1)
                        F = f8_pass()
                        drain(0, ps0, F)
                        drain(1, ps1_, F)

            _psum_cm.__exit__(None, None, None)

    nc.compile()
    return nc


def _prep_inputs(input1, input2, W):
    """Host-side shard + layout (transposes / gathers / dtype casts only)."""
    x1 = np.ascontiguousarray(input1, dtype=np.float32)
    x2 = np.ascontiguousarray(input2, dtype=np.float32)
    Wt = np.ascontiguousarray(W.T, dtype=np.float32)  # [65536, 1024], k-major

    in_maps = []
    for core in range(NCORES):
        ks = slice(core * KL, (core + 1) * KL)
        gs = slice(core * GPC, (core + 1) * GPC)
        Wk = Wt[ks].reshape(NCHUNK, 128, O)
        # bf16 chunks 0..NBF-1: [c, p, o] -> [p, c*O + o]
        wp = (
            Wk[:NBF]
            .transpose(1, 0, 2)
            .reshape(128, NBF * O)
            .astype(BF16)
        )
        # fp8 chunks NBF..63, DoubleRow pairs: [p, j, s, o], scaled 2^SW8
        wf = (
            (Wk[NBF:] * float(2.0 ** SW8))
            .reshape(NF8, 2, 128, O)
            .transpose(2, 0, 1, 3)
            .astype(F8)
        )
        # x1 replicated over j: k_local = g*B*B + i*B + j -> x1[l, g, i]
        x1g = x1.reshape(L, G, B)[:, gs, :].transpose(1, 2, 0)  # [g, i, l]
        rep = np.repeat(x1g, B, axis=1).reshape(KL, L)          # [k_local, l]
        x1r = (
            rep.reshape(NCHUNK, 128, NLB, LB)
            .transpose(2, 1, 0, 3)
            .reshape(NLB, 128, KL)
            .astype(BF16)
        )
        # x2 stacked twice along partitions: row p -> j = p % 64
        x2g = x2.reshape(L, G, B)[:, gs, :].transpose(1, 2, 0)  # [g, j, l]
        x2st = np.concatenate([x2g, x2g], axis=1).astype(BF16)  # [g, 128, l]
        hd0 = np.concatenate(
            [x1r[r, :, 0:128] for r in range(R1)] + [x2st[0][:, 0:R1 * 128]],
            axis=1,
        )
        hd1 = np.concatenate(
            [x1r[r, :, 128:640] for r in range(R1)] + [x2st[1][:, 0:R1 * 128]],
            axis=1,
        )
        in_maps.append(
            {
                "wp": np.ascontiguousarray(wp),
                "wf": np.ascontiguousarray(wf),
                "x1r": np.ascontiguousarray(x1r),
                "x2s": np.ascontiguousarray(x2st),
                "hd0": np.ascontiguousarray(hd0),
                "hd1": np.ascontiguousarray(hd1),
            }
        )
    return in_maps


def run(input1, input2, W, b, trace=False, tmpdir=None):
    """Shard, run on 8 NeuronCores, unshard. Returns (out, BassKernelResults)."""
    from concourse.bass_utils import run_bass_kernel_spmd

    if "nc" not in _cache:
        _cache["nc"] = _build_nc()
    nc = _cache["nc"]

    in_maps = _prep_inputs(input1, input2, W)
    res = run_bass_kernel_spmd(
        nc, in_maps, list(range(NCORES)), trace=trace, tmpdir=tmpdir
    )
    acc = np.zeros((L, O), dtype=np.float32)
    for core in range(NCORES):
        acc += res.results[core]["out"].astype(np.float32)
    acc += np.asarray(b, dtype=np.float32)[None, :]
    return acc, res


def kernel(input1, input2, W, b):
    out, _ = run(input1, input2, W, b, trace=False)
    return out


if __name__ == "__main__":
    rng = np.random.default_rng(0)
    x1 = rng.standard_normal((L, H), dtype=np.float32)
    x2 = rng.standard_normal((L, H), dtype=np.float32)
    W = rng.standard_normal((O, H * B), dtype=np.float32) / 256.0
    b = rng.standard_normal((O,), dtype=np.float32) / 256.0
    out = kernel(x1, x2, W, b)
    print("out", out.shape, out.dtype, float(np.abs(out).max()))
